# revision 8
# baseline (speedup 1.0000x reference)
"""Trainium2 Bass kernel: cached causal self-attention (dense transformer block).

Full module: y = CausalAttn(x; Wq, Wk, Wv) @ Wo.T + bo with
  B=4, S=2048, E=2048, H=16 heads, Dh=128, fp32 inputs.

Distribution: 8-way tensor parallel over heads (2 heads per NeuronCore).
Each core computes Q/K/V projections for its 2 heads (contraction over the
full embedding dim), causal-softmax attention for those heads, and a partial
output projection y_c = ctx_c @ Wo[:, c*256:(c+1)*256].T.  The host sums the
8 partials and adds the bias, avoiding on-device collectives.

v7 vs v6 (~720 us):
  - Scores are software-pipelined one k-tile ahead of the attn@V matmuls
    (3 score PSUM tiles), so every exp chain has a full k-tile (~1 us) of
    independent PE work behind it and the AV matmuls never wait on
    mask->exp latency.  This replaces the projection-task-as-filler trick
    inside the k-loop, which poisoned the ACT queue: each drained task's
    PSUM eviction sat between exp ops and delayed the next chain ~250 ns.
  - The softmax denominator is computed by a single Pool-engine
    partition_all_reduce on the exp accumulator (replacing the ones-matmul
    + [1,512] eviction + partition_broadcast), freeing the PE, the ACT
    queue, and a PSUM bank -- which funds the third score tile.
  - Attention is ACT-bound (exp is ~44 us/batch vs ~36 us of PE work), so
    projection tasks are split: ~half drain inside the attention k-loop
    (free PE filler under the ACT ceiling), the rest drain during phase A
    chunks (where ACT is nearly idle).  The last batch drains everything
    in-loop since no phase A follows.
  - Exp accumulation: head 0 on DVE, head 1 on Pool (DVE alone saturated
    against the ~0.9 us stall-free k-tile cadence).

v6: startup DMAs in first-use order + Q/K head-interleaved by e-tile half
(~10 us); phase-granular task aging.  v5: reciprocal_approx_fast (exact
[128,512] reciprocal is 3.3 us).  v4: normalize ctx before the projection;
both heads accumulate into one PSUM tile, single plain eviction.  v3:
fine-grained projection drain, heads interleaved per k-tile, [128,1024] y
stores from the sync engine.  v2: everything bf16, denominator off the PE,
mask add only on the 128-wide diagonal block.

Layout: x pre-transposed on host (xT [E, B*S]); scores computed transposed
(sT[k, q]) so exp(sT) feeds the attn@V matmul as the moving operand.
"""

import math

import ml_dtypes
import numpy as np

import concourse.bacc as bacc
import concourse.bass_isa as bass_isa
import concourse.mybir as mybir
import concourse.tile as tile
from concourse.bass_utils import run_bass_kernel_spmd

F32 = mybir.dt.float32
F32R = mybir.dt.float32r
BF16 = mybir.dt.bfloat16
AF = mybir.ActivationFunctionType
ALU = mybir.AluOpType

NEG = -1.0e30

# Full-problem constants
EMB = 2048
N_HEADS = 16
HEAD_DIM = 128
B_FULL = 4
S_FULL = 2048
N_CORES = 8
HPC = N_HEADS // N_CORES  # heads per core = 2


def build(B=B_FULL, S=S_FULL, E=EMB, hpc=HPC, DH=HEAD_DIM, CH=512, reps=1):
    """Build the per-core Bass program (same program on all 8 cores)."""
    assert hpc == 2, "y eviction chain is written for 2 heads per core"
    SB = B * S
    DHC = hpc * DH          # per-core head dims (256)
    NE = E // 128           # e-tiles (contraction tiles)
    NCH = S // CH           # 512-wide chunks per sequence
    KPC = CH // 128         # k-tiles per chunk (4)
    NST = S // 128          # 128-row s-tiles per sequence
    NOC = E // CH           # output chunks
    scale = 1.0 / math.sqrt(DH)

    nc = bacc.Bacc("TRN2", target_bir_lowering=False, debug=False,
                   num_devices=N_CORES)

    xT = nc.dram_tensor("xT", [E, SB], BF16, kind="ExternalInput")
    wqT = nc.dram_tensor("wqT", [E, DHC], BF16, kind="ExternalInput")
    wkT = nc.dram_tensor("wkT", [E, DHC], BF16, kind="ExternalInput")
    wvT = nc.dram_tensor("wvT", [E, DHC], BF16, kind="ExternalInput")
    woT = nc.dram_tensor("woT", [DHC, E], BF16, kind="ExternalInput")
    masks = nc.dram_tensor("masks", [128, 128], BF16, kind="ExternalInput")
    y = nc.dram_tensor("y", [SB, E], BF16, kind="ExternalOutput")

    with tile.TileContext(nc) as tc:
        with (
            tc.tile_pool(name="wpool", bufs=1) as wpool,
            tc.tile_pool(name="xtp", bufs=2) as xtp,
            tc.tile_pool(name="qkv", bufs=1) as qkv,
            tc.tile_pool(name="expp", bufs=4) as expp,
            tc.tile_pool(name="exac", bufs=2) as exac,
            tc.tile_pool(name="denp", bufs=2) as denp_sb,
            tc.tile_pool(name="yout", bufs=3) as yout,
            tc.tile_pool(name="ps_mm", bufs=3, space="PSUM") as ps_mm,
            tc.tile_pool(name="ps_proj", bufs=3, space="PSUM") as ps_proj,
            tc.tile_pool(name="ps_av", bufs=2, space="PSUM") as ps_av,
        ):
            # Resident weights / constants, DMA'd in first-use order so the
            # PE can start as soon as the first ~1.5 MB lands.
            wq_sb = wpool.tile([128, NE, DHC], BF16, tag="wq")
            wk_sb = wpool.tile([128, NE, DHC], BF16, tag="wk")
            wv_sb = wpool.tile([128, NE, DHC], BF16, tag="wv")
            wo_sb = wpool.tile([128, hpc, E], BF16, tag="wo")
            xT_r = xT.rearrange("(t p) s -> p t s", p=128)
            NEH = NE // 2
            wq_r = wqT.rearrange("(t p) d -> p t d", p=128)
            wk_r = wkT.rearrange("(t p) d -> p t d", p=128)
            nc.sync.dma_start(wq_sb[:, 0:NEH, :], wq_r[:, 0:NEH, :])
            xpre0 = None
            if reps == 1:
                x0a = xtp.tile([128, NEH, CH], BF16, tag="xta", name="x0a")
                nc.sync.dma_start(x0a[:], xT_r[:, 0:NEH, 0:CH])
                xpre0 = ((0, 0), x0a, None)
            nc.sync.dma_start(wq_sb[:, NEH:NE, :], wq_r[:, NEH:NE, :])
            if reps == 1:
                x0b = xtp.tile([128, NEH, CH], BF16, tag="xtb", name="x0b")
                nc.sync.dma_start(x0b[:], xT_r[:, NEH:NE, 0:CH])
                xpre0 = ((0, 0), x0a, x0b)
            nc.sync.dma_start(wk_sb[:, 0:NEH, :], wk_r[:, 0:NEH, :])
            nc.sync.dma_start(wk_sb[:, NEH:NE, :], wk_r[:, NEH:NE, :])
            nc.sync.dma_start(wv_sb[:], wvT.rearrange("(t p) d -> p t d", p=128))
            nc.sync.dma_start(wo_sb[:], woT.rearrange("(h p) e -> p h e", p=128))
            mask_sb = wpool.tile([128, 128], BF16, tag="mask")
            nc.sync.dma_start(mask_sb[:], masks[:, :])

            # ---- fine-grained output-projection task queue -------------
            # A task covers one s-tile x two 512-wide output chunks: both
            # heads' matmuls accumulate into one PSUM tile per chunk (ctx is
            # already normalized), one plain eviction each (alternating
            # ACT/DVE; Pool cannot read PSUM), one [128, 1024] y DMA from
            # the sync engine.  Tasks age at least one phase (gen bumps at
            # phase A and at every attention chunk) before being drained.
            pending = []
            evct = [0]
            gen = [0]

            def queue_proj(pctx, ps0, g):
                for st in range(g * KPC, (g + 1) * KPC):
                    for oc2 in range(NOC // 2):
                        pending.append((gen[0], pctx, ps0, st, oc2))

            def emit_one():
                _, pctx, ps0, st, oc2 = pending.pop(0)
                ysb = yout.tile([128, 2 * CH], BF16, tag="ysb")
                for i in range(2):
                    oc = oc2 * 2 + i
                    o0 = oc * CH
                    p = ps_proj.tile([128, CH], F32, tag="proj")
                    nc.tensor.matmul(
                        p[:], pctx[:, 0, st * 128:(st + 1) * 128],
                        wo_sb[:, 0, o0:o0 + CH], start=True, stop=False)
                    nc.tensor.matmul(
                        p[:], pctx[:, 1, st * 128:(st + 1) * 128],
                        wo_sb[:, 1, o0:o0 + CH], start=False, stop=True)
                    ys = ysb[:, i * CH:(i + 1) * CH]
                    if evct[0] % 2 == 0:
                        nc.scalar.activation(ys, p[:], AF.Identity)
                    else:
                        nc.vector.tensor_copy(ys, p[:])
                    evct[0] += 1
                nc.sync.dma_start(
                    y[ps0 + st * 128:ps0 + (st + 1) * 128,
                      oc2 * 2 * CH:(oc2 + 1) * 2 * CH], ysb[:])

            def drain(n, minage=2):
                for _ in range(n):
                    if not pending:
                        return
                    if pending[0][0] > gen[0] - minage:
                        return
                    emit_one()

            import contextlib
            rep_cm = tc.For_i(0, reps, 1) if reps > 1 else contextlib.nullcontext()
            with rep_cm:
              for b in range(B):
                  s0 = b * S
                  gen[0] += 1
                  # ---------------- Phase A: Q/K/V projections -------------
                  qT = qkv.tile([128, hpc, S], BF16, tag="qT")
                  kT = qkv.tile([128, hpc, S], BF16, tag="kT")
                  v_sb = qkv.tile([128, NST, DHC], BF16, tag="v")
                  if b == 0:
                      xpre = xpre0
                  for ch in range(NCH):
                      c0 = ch * CH
                      if xpre is not None and xpre[0] == (b, ch):
                          xta, xtb = xpre[1], xpre[2]
                      else:
                          xta = xtp.tile([128, NEH, CH], BF16, tag="xta")
                          nc.sync.dma_start(xta[:],
                                            xT_r[:, 0:NEH, s0 + c0:s0 + c0 + CH])
                          xtb = xtp.tile([128, NEH, CH], BF16, tag="xtb")
                          nc.sync.dma_start(xtb[:],
                                            xT_r[:, NEH:NE, s0 + c0:s0 + c0 + CH])
                      if ch + 1 < NCH or b + 1 < B:
                          nb_, nch = (b, ch + 1) if ch + 1 < NCH else (b + 1, 0)
                          n0 = nb_ * S + nch * CH
                          xna = xtp.tile([128, NEH, CH], BF16, tag="xta",
                                         name="xna")
                          nc.sync.dma_start(xna[:], xT_r[:, 0:NEH, n0:n0 + CH])
                          xnb = xtp.tile([128, NEH, CH], BF16, tag="xtb",
                                         name="xnb")
                          nc.sync.dma_start(xnb[:], xT_r[:, NEH:NE, n0:n0 + CH])
                          xpre = ((nb_, nch), xna, xnb)
                      else:
                          xpre = None

                      def xslice(et, lo=None, hi=None):
                          t = xta if et < NEH else xtb
                          e = et if et < NEH else et - NEH
                          if lo is None:
                              return t[:, e, :]
                          return t[:, e, lo:hi]

                      # Q then K, both heads interleaved by e-tile half so
                      # the first matmuls only need the low half of the
                      # weights + x chunk (startup)
                      for w_sb, dst, evscale in ((wq_sb, qT, scale),
                                                 (wk_sb, kT, 1.0)):
                          pp = [ps_mm.tile([128, CH], F32, tag="qkvp",
                                           name=f"pp{h}") for h in range(hpc)]
                          for half in range(2):
                              for h in range(hpc):
                                  for et in range(half * NEH,
                                                  half * NEH + NEH):
                                      nc.tensor.matmul(
                                          pp[h][:],
                                          w_sb[:, et, h * DH:(h + 1) * DH],
                                          xslice(et),
                                          start=(et == 0), stop=(et == NE - 1))
                              if half == 1:
                                  for h in range(hpc):
                                      nc.scalar.activation(
                                          dst[:, h, c0:c0 + CH], pp[h][:],
                                          AF.Identity, scale=evscale)
                          drain(2, minage=1)
                      for st in range(KPC):
                          vp = ps_mm.tile([128, DHC], F32, tag="qkvp")
                          for et in range(NE):
                              nc.tensor.matmul(
                                  vp[:], xslice(et, st * 128, (st + 1) * 128),
                                  wv_sb[:, et, :],
                                  start=(et == 0), stop=(et == NE - 1))
                          nc.scalar.activation(v_sb[:, ch * KPC + st, :], vp[:],
                                               AF.Identity)
                          if st % 2 == 1:
                              drain(1, minage=1)

                  # ------- Phase B: attention, scores pipelined 1 k-tile ahead --
                  ctxT = qkv.tile([128, hpc, S], BF16, tag="ctxT")
                  ctxn = qkv.tile([128, hpc, S], BF16, tag="ctxn")
                  for g in range(NCH):
                      gen[0] += 1
                      nk = KPC * (g + 1)
                      avp = [ps_av.tile([128, CH], F32, tag="av",
                                        name=f"avp{h}") for h in range(hpc)]
                      ex_acc = [exac.tile([128, CH], F32R, tag="exacc",
                                          name=f"exacc{h}") for h in range(hpc)]

                      def offs(kt):
                          j = kt - (nk - KPC)
                          return (128 * j if j > 0 else 0), j

                      def emit_score(h, kt):
                          off, j = offs(kt)
                          sp = ps_mm.tile([128, CH], F32, tag="qkvp",
                                          name="sp")
                          nc.tensor.matmul(
                              sp[:, off:], kT[:, h, kt * 128:(kt + 1) * 128],
                              qT[:, h, g * CH + off:(g + 1) * CH],
                              start=True, stop=True)
                          if j >= 0:
                              # mask col c: masked iff c < p (strict tri);
                              # columns past the diagonal block are never
                              # masked
                              nc.vector.tensor_add(sp[:, off:off + 128],
                                                   sp[:, off:off + 128],
                                                   mask_sb[:, :])
                          ex = expp.tile([128, CH], BF16, tag="ex")
                          nc.scalar.activation(ex[:, off:], sp[:, off:],
                                               AF.Exp)
                          return ex

                      exq = {(h, 0): emit_score(h, 0) for h in range(hpc)}
                      for kt in range(nk):
                          off, _ = offs(kt)
                          exs = []
                          for h in range(hpc):
                              if kt + 1 < nk:
                                  exq[(h, kt + 1)] = emit_score(h, kt + 1)
                              ex = exq.pop((h, kt))
                              exs.append(ex)
                              nc.tensor.matmul(
                                  avp[h][:, off:],
                                  v_sb[:, kt, h * DH:(h + 1) * DH],
                                  ex[:, off:],
                                  start=(kt == 0), stop=(kt == nk - 1),
                                  skip_group_check=True)
                          # denominator accumulation off the PE: DVE takes
                          # head 0, the Pool engine head 1
                          for h in range(hpc):
                              eng = nc.vector if h == 0 else nc.gpsimd
                              if kt == 0:
                                  eng.tensor_copy(ex_acc[h][:, :], exs[h][:, :])
                              else:
                                  eng.tensor_add(ex_acc[h][:, off:],
                                                 ex_acc[h][:, off:],
                                                 exs[h][:, off:])
                          if b == B - 1:
                              if kt >= 4:
                                  drain(2, minage=1)
                              elif kt >= 2:
                                  drain(1, minage=2)
                          elif kt >= 2:
                              drain(1, minage=2)
                      for h in range(hpc):
                          gs = slice(g * CH, (g + 1) * CH)
                          nc.scalar.activation(ctxT[:, h, gs], avp[h][:],
                                               AF.Identity)
                          # denominator row broadcast to all partitions in
                          # one Pool op, then reciprocal + multiply on DVE
                          rbc = denp_sb.tile([128, CH], F32, tag="rbc")
                          nc.gpsimd.partition_all_reduce(
                              rbc[:], ex_acc[h][:], channels=128,
                              reduce_op=bass_isa.ReduceOp.add)
                          nc.vector.reciprocal_approx_fast(rbc[:], rbc[:])
                          nc.vector.tensor_mul(ctxn[:, h, gs], ctxT[:, h, gs],
                                               rbc[:])
                      queue_proj(ctxn, s0, g)
              for _ in range(len(pending)):
                  emit_one()
    nc.finalize()
    return nc


def host_consts(S=S_FULL, CH=512):
    """Causal mask constant input."""
    p = np.arange(128)[:, None]
    c = np.arange(128)[None, :]
    # strict lower triangle: masked iff c < p (diagonal k-tile suffix mask)
    masks = np.where(c < p, np.float32(NEG), np.float32(0.0))
    masks = np.ascontiguousarray(masks.astype(ml_dtypes.bfloat16))
    return {"masks": masks}


def host_inputs(x, Wq, Wk, Wv, Wo, B=B_FULL, S=S_FULL, E=EMB, hpc=HPC,
                DH=HEAD_DIM, CH=512):
    """Shard + lay out the full inputs for the 8 cores."""
    SB = B * S
    DHC = hpc * DH
    bf = ml_dtypes.bfloat16
    xT = np.ascontiguousarray(x.reshape(SB, E).T.astype(bf))
    consts = host_consts(S, CH)

    in_maps = []
    for c in range(N_CORES):
        lo, hi = c * DHC, (c + 1) * DHC
        in_maps.append({
            "xT": xT,
            "wqT": np.ascontiguousarray(Wq[lo:hi, :].T.astype(bf)),
            "wkT": np.ascontiguousarray(Wk[lo:hi, :].T.astype(bf)),
            "wvT": np.ascontiguousarray(Wv[lo:hi, :].T.astype(bf)),
            "woT": np.ascontiguousarray(Wo[:, lo:hi].T.astype(bf)),
            **consts,
        })
    return in_maps


def kernel(x, Wq, Wk, Wv, Wo, bo):
    x = np.asarray(x, dtype=np.float32)
    Wq = np.asarray(Wq, dtype=np.float32)
    Wk = np.asarray(Wk, dtype=np.float32)
    Wv = np.asarray(Wv, dtype=np.float32)
    Wo = np.asarray(Wo, dtype=np.float32)
    bo = np.asarray(bo, dtype=np.float32)

    nc = build()
    in_maps = host_inputs(x, Wq, Wk, Wv, Wo)
    res = run_bass_kernel_spmd(nc, in_maps, list(range(N_CORES)))
    y = res.results[0]["y"].astype(np.float64)
    for c in range(1, N_CORES):
        y += res.results[c]["y"].astype(np.float64)
    y = (y + bo).astype(np.float32)
    return y.reshape(B_FULL, S_FULL, EMB)
